# revision 1
# baseline (speedup 1.0000x reference)
"""CenterLoss kernel for Trainium2 (8 NeuronCores, SPMD data-parallel).

Math: for pixel p with feature x_p (256-ch), label l_p, centers C[19,256]:
    dist_p = ||xn_p||^2 + ||cn_{l_p}||^2 - 2 * xn_p . cn_{l_p}
with xn = x/||x||, cn = C/||C|| (row-wise).  ||xn||^2 == ||cn||^2 == 1 up to
f32 rounding (~1e-7, negligible vs the O(1) mean), so
    mean(dist) = 2 - (2/B) * S,   S = sum_p (x_p . cn_{l_p}) / ||x_p||.

Device computes S (everything except the final 8-way scalar sum):
  - dots[19,pix]  = cnT.T @ x           (PE, f32)
  - mask[19,pix]  = (lab==k) * rc_k     (DVE tensor_scalar, rc = 1/||C_k||)
  - prodsel       = mask * dots         (DVE, bf16 out)
  - sel_col[128,1] per 128-pixel group = prodsel_group.T @ ones  (PE)
  - ss_col[128,1]  per group           = xsq_group.T @ ones      (PE)
  - partial[p] = sum_g sel[p,g] / sqrt(ss[p,g])  (ACT sqrt + DVE recip + DVE ttr)

Sharding: 65536 pixels -> 8 cores x 8192 (core c: image c//2, half c%2).
x shipped channel-major [2,128,8192] per core; centersT + labels replicated
per shard; labels pre-cast to f32 (exact for values < 19).
"""

import sys

import numpy as np

if "/opt/trn_rl_repo" not in sys.path:
    sys.path.insert(0, "/opt/trn_rl_repo")

import concourse.bacc as bacc
import concourse.bass as bass
import concourse.tile as tile
from concourse import mybir
from concourse.bass_utils import run_bass_kernel_spmd

N_CORES = 8
C = 256
NCLS = 19
N_IMG, H, W = 4, 128, 128
PIX_TOTAL = N_IMG * H * W          # 65536
PIX_PER_CORE = PIX_TOTAL // N_CORES  # 8192
TILE_F = 2048                      # pixels per DMA tile (8KB descriptors)
N_TILES = PIX_PER_CORE // TILE_F   # 4
N_PAIRS = PIX_PER_CORE // 1024     # 8 (a "pair" = 2 halves = 1024 px)
HALF = 512                         # fp32 matmul max moving free dim
F32 = mybir.dt.float32
BF16 = mybir.dt.bfloat16


def build_nc():
    """Build the per-core Bass program (same program on all 8 cores)."""
    AF = mybir.ActivationFunctionType
    OP = mybir.AluOpType

    import ml_dtypes

    # Bacc (not raw Bass): its compile() runs generate_event_semaphores,
    # which legalizes multi-wait instructions down to the TRN2 limit of one
    # sync-wait per instruction (walrus hard-errors otherwise).
    nc = bacc.Bacc(None, target_bir_lowering=False, debug=False)
    x_d = nc.dram_tensor("x", [2, 128, PIX_PER_CORE], F32, kind="ExternalInput")
    lab_d = nc.dram_tensor(
        "labels", [NCLS, PIX_PER_CORE], F32, kind="ExternalInput"
    )
    ct_d = nc.dram_tensor("centersT", [2, 128, NCLS], F32, kind="ExternalInput")
    out_d = nc.dram_tensor("out", [128, 1], F32, kind="ExternalOutput")
    iota_d = nc.inline_tensor(
        np.arange(NCLS, dtype=np.float32).reshape(NCLS, 1), name="iota19"
    )
    ident_d = nc.inline_tensor(
        np.eye(128, dtype=ml_dtypes.bfloat16), name="ident128"
    )

    with tile.TileContext(nc) as tc:
        with (
            tc.tile_pool(name="consts", bufs=1) as consts,
            tc.tile_pool(name="xin", bufs=4) as xin,
            tc.tile_pool(name="xsq", bufs=3) as xsqp,
            tc.tile_pool(name="small", bufs=2) as small,
            tc.tile_pool(name="accum", bufs=1) as accp,
            tc.tile_pool(name="dots", bufs=2, space="PSUM") as dotsp,
            tc.tile_pool(name="rows", bufs=1, space="PSUM") as rowsp,
            tc.tile_pool(name="tpp", bufs=3, space="PSUM") as tpp,
        ):
            # ---- constants ----
            # Wait-funnel discipline: walrus allows only ONE sync-wait per
            # Matmult, so every matmul's operands must be reachable through
            # a single semaphore at its issue point.  All matmul-visible
            # constants are produced by the DVE (copy/memset), and one dummy
            # matmul at the end of setup makes PE observe the latest DVE
            # tick before the main loop.
            ct_in = consts.tile([128, 2, NCLS], F32, tag="ct_in")
            nc.sync.dma_start(out=ct_in[:], in_=ct_d[:].rearrange("a p k -> p a k"))
            ct = consts.tile([128, 2, NCLS], F32, tag="ct")
            nc.vector.tensor_copy(ct[:], ct_in[:])
            ones_f = consts.tile([128, 1], F32, tag="ones_f")
            nc.vector.memset(ones_f[:], 1.0)
            ones_b = consts.tile([128, 1], BF16, tag="ones_b")
            nc.vector.memset(ones_b[:], 1.0)
            # rcb: per-class 1/||C_k|| as the sel-matmul moving operand
            # (rows 19..127 zero); filled after rc is computed below.
            rcb = consts.tile([128, 1], BF16, tag="rcb")
            nc.vector.memset(rcb[:], 0.0)
            iota_in = consts.tile([NCLS, 1], F32, tag="iota_in")
            nc.sync.dma_start(out=iota_in[:], in_=iota_d[:])
            # pre-read iota on DVE so per-tile mask ops never wait on its DMA
            iota = consts.tile([NCLS, 1], F32, tag="iota")
            nc.vector.tensor_copy(iota[:], iota_in[:])
            ident_in = consts.tile([128, 128], BF16, tag="ident_in")
            nc.sync.dma_start(out=ident_in[:], in_=ident_d[:])
            ident = consts.tile([128, 128], BF16, tag="ident")
            nc.vector.tensor_copy(ident[:], ident_in[:])
            # all labels up front, via the GPSIMD SW-DGE: a [19, F] DMA only
            # engages the HW engines serving partitions 0-18 AND the HW ring
            # is in-order, so putting labels there stalls the x stream.
            # 4 SWDGE queues load them concurrently off the critical ring.
            labb_all = consts.tile([NCLS, PIX_PER_CORE], F32, tag="labb_all")
            for q in range(4):
                qs = slice(q * (PIX_PER_CORE // 4), (q + 1) * (PIX_PER_CORE // 4))
                nc.gpsimd.dma_start(out=labb_all[:, qs], in_=lab_d[:, qs])

            # ss/sel rows psum tiles (three, rotating, so each is a stable
            # slot we can zero-fill once: unwritten partitions stay 0
            # instead of stale PSUM garbage).
            rows_ps = []
            for i in range(3):
                rp = rowsp.tile([128, HALF], F32, tag=f"rows{i}")
                nc.vector.memset(rp[:], 0.0)
                rows_ps.append(rp)

            # prodsel buffers: padded to K=128 with zero rows so the sel
            # matmul contracts over the full partition range (and bf16 gets
            # FWL on the weight load).  bufs=1 pool + distinct tags => each
            # is a persistent slot, zero rows stay zero.  4 slots (half x
            # pair-parity) keep the DVE write 2 pairs ahead of the PE read.
            prodsel = []
            for i in range(4):
                pst = accp.tile([128, HALF], BF16, tag=f"prodsel{i}")
                nc.vector.memset(pst[:], 0.0)
                prodsel.append(pst)

            # csq is the LAST DVE setup op: the ssc matmuls' single DVE wait
            # then covers every DVE-produced constant above (ct, ones, zero
            # fills), so later matmuls never need a second wait for them.
            csq = consts.tile([128, 2, NCLS], F32, tag="csq")
            nc.vector.tensor_mul(out=csq[:], in0=ct[:], in1=ct[:])

            # ---- center norms: rc[k] = 1/||C_k|| ----
            ssc = dotsp.tile([NCLS, 1], F32, tag="dots")
            nc.tensor.matmul(ssc[:], csq[:, 0, :], ones_f[:], start=True, stop=False)
            nc.tensor.matmul(ssc[:], csq[:, 1, :], ones_f[:], start=False, stop=True)
            # rc = 1/sqrt(ssc) via ACT Sqrt + DVE reciprocal (Rsqrt is banned
            # on ACT; tensor_tensor_reduce faults on this runtime)
            rc = consts.tile([NCLS, 1], F32, tag="rc")
            nc.scalar.activation(out=rc[:], in_=ssc[:], func=AF.Sqrt)
            nc.vector.reciprocal(out=rc[:], in_=rc[:])
            nc.vector.tensor_copy(rcb[0:NCLS, :], rc[:])

            # PE warm-up: ~4us of dummy transposes while the first x tiles
            # stream in, so HAM un-throttles the PE clock (1.2 -> 2.4 GHz)
            # before real matmuls start.
            warm = tpp.tile([128, 128], BF16, tag="tp")
            for _ in range(36):
                nc.tensor.transpose(warm[:], ident[:], ident[:])

            # transposed per-pair tiles, combined once after the loop
            tp_all = accp.tile([128, N_PAIRS, 4, 128], BF16, tag="tp_all")

            # ---- main loop over DMA tiles (2048 px each) ----
            # ss and sel are computed as PSUM *rows* by big N=512 matmuls
            # placed into distinct tile_position col-groups (partitions
            # 0/32/64/96 of one PSUM tile), then a PE transpose flips them to
            # pixel-on-partition layout for the cheap partition-parallel
            # sqrt/recip/mult finish.  The x load is split into 4 dma_starts
            # (2 chunks x 2 partition halves) so all 16 DMA engines stream
            # concurrently across the 4 buffered tiles.
            for t in range(N_TILES):
                fsl = slice(t * TILE_F, (t + 1) * TILE_F)
                xt = xin.tile([128, 2, TILE_F], F32, tag="xt")
                for a in range(2):
                    nc.sync.dma_start(out=xt[:, a, :], in_=x_d[a, :, fsl])

                xsqt = xsqp.tile([128, 2, TILE_F], BF16, tag="xsqt")
                for a in range(2):
                    nc.scalar.activation(
                        out=xsqt[:, a, :], in_=xt[:, a, :], func=AF.Square
                    )

                # mask[k, p] = (lab_p == k); the rc_k scale is folded into
                # the sel matmul's stationary operand (rcb)
                mask = small.tile([NCLS, TILE_F], F32, tag="mask")
                nc.vector.tensor_scalar(
                    out=mask[:],
                    in0=labb_all[:, fsl],
                    scalar1=iota[:],
                    scalar2=None,
                    op0=OP.is_equal,
                )

                for pr in range(2):
                    pg_idx = 2 * t + pr
                    rows = rows_ps[pg_idx % 3]
                    for hh in range(2):
                        h = 2 * pr + hh
                        hsl = slice(h * HALF, (h + 1) * HALF)
                        dots = dotsp.tile([NCLS, HALF], F32, tag="dots")
                        nc.tensor.matmul(
                            dots[:], ct[:, 0, :], xt[:, 0, hsl],
                            start=True, stop=False,
                        )
                        nc.tensor.matmul(
                            dots[:], ct[:, 1, :], xt[:, 1, hsl],
                            start=False, stop=True,
                        )
                        ps_i = 2 * (pg_idx % 2) + hh
                        nc.vector.tensor_mul(
                            out=prodsel[ps_i][0:NCLS, :],
                            in0=mask[:, hsl],
                            in1=dots[:],
                        )
                        # ss row for half hh -> col-group hh (partition 32hh)
                        nc.tensor.matmul(
                            rows[32 * hh : 32 * hh + 1, :],
                            ones_b[:],
                            xsqt[:, 0, hsl],
                            start=True,
                            stop=False,
                            tile_position=(0, 32 * hh),
                        )
                        nc.tensor.matmul(
                            rows[32 * hh : 32 * hh + 1, :],
                            ones_b[:],
                            xsqt[:, 1, hsl],
                            start=False,
                            stop=True,
                            tile_position=(0, 32 * hh),
                        )
                        # sel row (rc-scaled) for half hh -> col-group 2+hh
                        nc.tensor.matmul(
                            rows[64 + 32 * hh : 65 + 32 * hh, :],
                            rcb[:],
                            prodsel[ps_i][:],
                            start=True,
                            stop=True,
                            tile_position=(0, 64 + 32 * hh),
                        )

                    # rows -> SBUF (bf16), then PE-transpose 128x128 blocks
                    # so pixels land on partitions; kinds sit at free cols
                    # 0/32/64/96
                    rows_sb = small.tile([128, HALF], BF16, tag="rows_sb")
                    if pg_idx % 2 == 0:
                        nc.scalar.activation(
                            out=rows_sb[:], in_=rows[:], func=AF.Copy
                        )
                    else:
                        nc.vector.tensor_copy(rows_sb[:], rows[:])
                    tp = tpp.tile([128, 4, 128], BF16, tag="tp")
                    for j in range(4):
                        nc.tensor.transpose(
                            tp[:, j, :],
                            rows_sb[:, j * 128 : (j + 1) * 128],
                            ident[:],
                        )
                    # tp[p, j, b]: b=0 -> ss(h0), b=32 -> ss(h1),
                    # b=64 -> sel(h0), b=96 -> sel(h1);
                    # pixel index = hh*512 + j*128 + p within the pair.
                    # Stash the whole transposed tile; the sqrt/recip/mult
                    # finish runs once, after the loop.
                    nc.vector.tensor_copy(tp_all[:, pg_idx, :, :], tp[:])

            # ---- finalize: partial[p] = sum over pairs of sel/sqrt(ss) ----
            base = tp_all[:]
            ss_v = bass.AP(
                tensor=base.tensor,
                offset=base.offset,
                ap=[base.ap[0], [512, N_PAIRS], [128, 4], [32, 2]],
            )
            sel_v = bass.AP(
                tensor=base.tensor,
                offset=base.offset + 64,
                ap=[base.ap[0], [512, N_PAIRS], [128, 4], [32, 2]],
            )
            rsq = accp.tile([128, N_PAIRS, 4, 2], F32, tag="rsq")
            nc.scalar.activation(out=rsq[:], in_=ss_v, func=AF.Sqrt)
            nc.vector.reciprocal(out=rsq[:], in_=rsq[:])
            acc = accp.tile([128, N_PAIRS, 4, 2], F32, tag="acc")
            nc.vector.tensor_mul(out=acc[:], in0=sel_v, in1=rsq[:])
            partial = accp.tile([128, 1], F32, tag="partial")
            nc.vector.tensor_reduce(
                out=partial[:],
                in_=acc[:].rearrange("p a b c -> p (a b c)"),
                axis=mybir.AxisListType.X,
                op=mybir.AluOpType.add,
            )
            nc.sync.dma_start(out=out_d[:], in_=partial[:])

    nc.compile()
    return nc


def shard_inputs(x, centers, labels):
    """Full inputs -> list of 8 per-core input maps."""
    x = np.ascontiguousarray(np.asarray(x, dtype=np.float32))
    centers = np.ascontiguousarray(np.asarray(centers, dtype=np.float32))
    labels = np.asarray(labels)

    xr = x.reshape(N_IMG, C, 2, PIX_PER_CORE)
    labr = labels.reshape(N_IMG, 2, PIX_PER_CORE).astype(np.float32)
    ctr = np.ascontiguousarray(centers.T).reshape(2, 128, NCLS)

    in_maps = []
    for core in range(N_CORES):
        n, j = core // 2, core % 2
        xs = np.ascontiguousarray(xr[n, :, j, :]).reshape(2, 128, PIX_PER_CORE)
        labb = np.ascontiguousarray(
            np.broadcast_to(labr[n, j].reshape(1, PIX_PER_CORE), (NCLS, PIX_PER_CORE))
        )
        in_maps.append({"x": xs, "labels": labb, "centersT": ctr})
    return in_maps


_NC_CACHE = {}


def _ensure_ntff_hook():
    """Register the axon NTFF profile hook if the optional antenv.axon_hooks
    module is absent from this image (bass_utils hard-imports it when
    trace=True)."""
    try:
        from antenv.axon_hooks import get_axon_ntff_profile_hook  # noqa: F401

        return
    except ImportError:
        pass
    import types

    import antenv

    mod = types.ModuleType("antenv.axon_hooks")
    state = {"hook": None}
    mod.set_axon_ntff_profile_hook = lambda h: state.__setitem__("hook", h)
    mod.get_axon_ntff_profile_hook = lambda: state["hook"]
    sys.modules["antenv.axon_hooks"] = mod
    antenv.axon_hooks = mod
    try:
        from trn_agent_boot.trn_boot import _ntff_profile_via_ctypes

        mod.set_axon_ntff_profile_hook(
            _ntff_profile_via_ctypes("/opt/axon/libaxon_pjrt.so")
        )
    except Exception:
        pass


def kernel(x, centers, labels, _profile=False):
    in_maps = shard_inputs(x, centers, labels)
    if _profile:
        _ensure_ntff_hook()
    if "nc" not in _NC_CACHE:
        _NC_CACHE["nc"] = build_nc()
    nc = _NC_CACHE["nc"]
    res = run_bass_kernel_spmd(
        nc, in_maps, list(range(N_CORES)), trace=bool(_profile)
    )
    s = 0.0
    for r in res.results:
        s += float(np.asarray(r["out"], dtype=np.float64).sum())
    val = np.array(np.float32(2.0 - 2.0 * s / PIX_TOTAL))
    if _profile:
        return val, res
    return val



# revision 2
# speedup vs baseline: 1.4775x; 1.4775x over previous
"""CenterLoss kernel for Trainium2 (8 NeuronCores, SPMD data-parallel).

Math: for pixel p with feature x_p (256-ch), label l_p, centers C[19,256]:
    dist_p = ||xn_p||^2 + ||cn_{l_p}||^2 - 2 * xn_p . cn_{l_p}
with xn = x/||x||, cn = C/||C|| (row-wise).  ||xn||^2 == ||cn||^2 == 1 up to
f32 rounding (~1e-7, negligible vs the O(1) mean), so
    mean(dist) = 2 - (2/B) * S,   S = sum_p (x_p . cn_{l_p}) / ||x_p||.

Device computes S (everything except the final 8-way scalar sum):
  - dots[19,512]  = ctT.T @ x  per 512-px half    (PE, bf16)
  - prodsel       = onehot * dots                 (DVE, bf16 out; onehot
                    is staged on host from labels)
  - sel_acc[16,512] += (rc x e_h).T @ prodsel     (PE; stationary column h
                    carries rc=1/||C_k||, other columns zero, so half h's
                    sel row lands on PSUM partition h and other rows get
                    zero added)
  - ss_acc[16,512] += (ones x e_h).T @ xsq        (PE; same trick)
  - out[h,p] = sel_acc/sqrt(ss_acc)  reduced over free dim -> [16,1]

Sharding: 65536 pixels -> 8 cores x 8192 (core c: image c//2, half c%2).
x shipped channel-major bf16 [2,128,8192]; onehot [19,8192] bf16 and
centersT bf16 replicated per shard.  Host sums the 16x8 partials.
"""

import sys

import numpy as np

if "/opt/trn_rl_repo" not in sys.path:
    sys.path.insert(0, "/opt/trn_rl_repo")

import concourse.bacc as bacc
import concourse.tile as tile
from concourse import mybir
from concourse.bass_utils import run_bass_kernel_spmd

N_CORES = 8
C = 256
NCLS = 19
N_IMG, H, W = 4, 128, 128
PIX_TOTAL = N_IMG * H * W          # 65536
PIX_PER_CORE = PIX_TOTAL // N_CORES  # 8192
TILE_F = 2048                      # pixels per DMA tile
N_TILES = PIX_PER_CORE // TILE_F   # 4
HALF = 512                         # psum bank free size (f32)
N_HALVES = PIX_PER_CORE // HALF    # 16
F32 = mybir.dt.float32
BF16 = mybir.dt.bfloat16


def build_nc():
    """Build the per-core Bass program (same program on all 8 cores)."""
    AF = mybir.ActivationFunctionType

    # Bacc (not raw Bass): its compile() runs generate_event_semaphores,
    # which legalizes multi-wait instructions down to the TRN2 limit of one
    # sync-wait per instruction (walrus hard-errors otherwise).
    nc = bacc.Bacc(None, target_bir_lowering=False, debug=False)
    x_d = nc.dram_tensor("x", [2, 128, PIX_PER_CORE], BF16, kind="ExternalInput")
    oh_d = nc.dram_tensor("onehot", [NCLS, PIX_PER_CORE], BF16, kind="ExternalInput")
    ct_d = nc.dram_tensor("centersT", [2, 128, NCLS], BF16, kind="ExternalInput")
    out_d = nc.dram_tensor("out", [N_HALVES, 1], F32, kind="ExternalOutput")

    with tile.TileContext(nc) as tc:
        with (
            tc.tile_pool(name="consts", bufs=1) as consts,
            tc.tile_pool(name="xin", bufs=3) as xin,
            tc.tile_pool(name="xsq", bufs=2) as xsqp,
            tc.tile_pool(name="small", bufs=3) as small,
            tc.tile_pool(name="accum", bufs=1) as accp,
            tc.tile_pool(name="dots", bufs=3, space="PSUM") as dotsp,
            tc.tile_pool(name="acc_ps", bufs=1, space="PSUM") as accps,
        ):
            # ---- constants (all matmul-visible constants DVE-produced so
            # matmuls wait through a single semaphore chain) ----
            ct_in = consts.tile([128, 2, NCLS], BF16, tag="ct_in")
            nc.sync.dma_start(out=ct_in[:], in_=ct_d[:].rearrange("a p k -> p a k"))
            ctb = consts.tile([128, 2, NCLS], BF16, tag="ctb")
            nc.vector.tensor_copy(ctb[:], ct_in[:])
            ones_b = consts.tile([128, 1], BF16, tag="ones_b")
            nc.vector.memset(ones_b[:], 1.0)
            # warmup moving operand
            wmov = consts.tile([128, HALF], BF16, tag="wmov")
            nc.vector.memset(wmov[:], 0.5)

            # ss stationary: sstat[:, h, :] is [128, 16] with column h all
            # ones, others zero -> half h's ones-contract lands on psum
            # partition h, zeros elsewhere (accumulate-safe).
            sstat = consts.tile([128, N_HALVES, N_HALVES], BF16, tag="sstat")
            nc.vector.memset(sstat[:], 0.0)
            for h in range(N_HALVES):
                nc.vector.memset(sstat[:, h, h : h + 1], 1.0)
            # sel stationary: selstat[0:19, h, :] column h = rc (filled after
            # rc is computed below), others zero.
            selstat = consts.tile([128, N_HALVES, N_HALVES], BF16, tag="selstat")
            nc.vector.memset(selstat[:], 0.0)

            # one-hot labels via GPSIMD SW-DGE queues, off the HW ring that
            # streams x.
            oh_all = consts.tile([NCLS, PIX_PER_CORE], BF16, tag="oh_all")
            for q in range(4):
                qs = slice(q * (PIX_PER_CORE // 4), (q + 1) * (PIX_PER_CORE // 4))
                nc.gpsimd.dma_start(out=oh_all[:, qs], in_=oh_d[:, qs])

            # ---- center norms: rc[k] = 1/||C_k|| ----
            csq = consts.tile([128, 2, NCLS], BF16, tag="csq")
            nc.vector.tensor_mul(out=csq[:], in0=ctb[:], in1=ctb[:])
            ssc = dotsp.tile([NCLS, 1], F32, tag="dots")
            nc.tensor.matmul(ssc[:], csq[:, 0, :], ones_b[:], start=True, stop=False)
            nc.tensor.matmul(ssc[:], csq[:, 1, :], ones_b[:], start=False, stop=True)
            rc = consts.tile([NCLS, 1], F32, tag="rc")
            nc.scalar.activation(out=rc[:], in_=ssc[:], func=AF.Sqrt)
            nc.vector.reciprocal(out=rc[:], in_=rc[:])
            for h in range(N_HALVES):
                nc.vector.tensor_copy(selstat[0:NCLS, h, h : h + 1], rc[:])

            # ---- PE warm-up: real matmuls (transposes don't engage HAM)
            # while the first x tiles stream in ----
            wscr = dotsp.tile([NCLS, HALF], F32, tag="dots")
            for _ in range(8):
                nc.tensor.matmul(wscr[:], ctb[:, 0, :], wmov[:], start=True, stop=True)

            # ---- accumulators ----
            ss_acc = accps.tile([N_HALVES, HALF], F32, tag="ss_acc")
            sel_acc = accps.tile([N_HALVES, HALF], F32, tag="sel_acc")

            # ---- main loop: 4 DMA tiles x 4 halves of 512 px ----
            # sel matmul for half h is emitted 2 halves late so the PE queue
            # never waits on the DVE prodsel of the current half.
            pending_sel = []

            def flush_sel(n):
                while len(pending_sel) > n:
                    h0, ps0 = pending_sel.pop(0)
                    nc.tensor.matmul(
                        sel_acc[:],
                        selstat[0:NCLS, h0, :],
                        ps0[:],
                        start=(h0 == 0),
                        stop=(h0 == N_HALVES - 1),
                    )

            for t in range(N_TILES):
                fsl = slice(t * TILE_F, (t + 1) * TILE_F)
                xt = xin.tile([128, 2, TILE_F], BF16, tag="xt")
                for a in range(2):
                    nc.sync.dma_start(out=xt[:, a, :], in_=x_d[a, :, fsl])
                xsqt = xsqp.tile([128, 2, TILE_F], BF16, tag="xsqt")
                for a in range(2):
                    nc.scalar.activation(
                        out=xsqt[:, a, :], in_=xt[:, a, :], func=AF.Square
                    )

                for hh in range(TILE_F // HALF):
                    h = t * (TILE_F // HALF) + hh
                    hsl = slice(hh * HALF, (hh + 1) * HALF)
                    gsl = slice(t * TILE_F + hh * HALF, t * TILE_F + (hh + 1) * HALF)

                    dots = dotsp.tile([NCLS, HALF], F32, tag="dots")
                    nc.tensor.matmul(
                        dots[:], ctb[:, 0, :], xt[:, 0, hsl], start=True, stop=False
                    )
                    nc.tensor.matmul(
                        dots[:], ctb[:, 1, :], xt[:, 1, hsl], start=False, stop=True
                    )
                    ps = small.tile([NCLS, HALF], BF16, tag="ps")
                    nc.vector.tensor_mul(out=ps[:], in0=oh_all[:, gsl], in1=dots[:])
                    pending_sel.append((h, ps))

                    nc.tensor.matmul(
                        ss_acc[:],
                        sstat[:, h, :],
                        xsqt[:, 0, hsl],
                        start=(h == 0),
                        stop=False,
                    )
                    nc.tensor.matmul(
                        ss_acc[:],
                        sstat[:, h, :],
                        xsqt[:, 1, hsl],
                        start=False,
                        stop=(h == N_HALVES - 1),
                    )
                    flush_sel(2)
            flush_sel(0)

            # ---- finalize: out[h] = sum_p sel/sqrt(ss) ----
            rsq = accp.tile([N_HALVES, HALF], F32, tag="rsq")
            nc.scalar.activation(out=rsq[:], in_=ss_acc[:], func=AF.Sqrt)
            nc.vector.reciprocal(out=rsq[:], in_=rsq[:])
            acc = accp.tile([N_HALVES, HALF], F32, tag="acc")
            nc.vector.tensor_mul(out=acc[:], in0=rsq[:], in1=sel_acc[:])
            partial = accp.tile([N_HALVES, 1], F32, tag="partial")
            nc.vector.tensor_reduce(
                out=partial[:],
                in_=acc[:],
                axis=mybir.AxisListType.X,
                op=mybir.AluOpType.add,
            )
            nc.sync.dma_start(out=out_d[:], in_=partial[:])

    nc.compile()
    return nc


def shard_inputs(x, centers, labels):
    """Full inputs -> list of 8 per-core input maps."""
    import ml_dtypes

    x = np.asarray(x, dtype=np.float32)
    centers = np.asarray(centers, dtype=np.float32)
    labels = np.asarray(labels)

    xr = x.reshape(N_IMG, C, 2, PIX_PER_CORE)
    labr = labels.reshape(N_IMG, 2, PIX_PER_CORE)
    ctr = np.ascontiguousarray(centers.T.astype(ml_dtypes.bfloat16)).reshape(
        2, 128, NCLS
    )
    kvals = np.arange(NCLS, dtype=np.int64).reshape(NCLS, 1)

    in_maps = []
    for core in range(N_CORES):
        n, j = core // 2, core % 2
        xs = np.ascontiguousarray(
            xr[n, :, j, :].astype(ml_dtypes.bfloat16)
        ).reshape(2, 128, PIX_PER_CORE)
        oh = (labr[n, j].reshape(1, PIX_PER_CORE) == kvals).astype(
            ml_dtypes.bfloat16
        )
        in_maps.append({"x": xs, "onehot": np.ascontiguousarray(oh), "centersT": ctr})
    return in_maps


_NC_CACHE = {}


def _ensure_ntff_hook():
    """Register the axon NTFF profile hook if the optional antenv.axon_hooks
    module is absent from this image (bass_utils hard-imports it when
    trace=True)."""
    try:
        from antenv.axon_hooks import get_axon_ntff_profile_hook  # noqa: F401

        return
    except ImportError:
        pass
    import types

    import antenv

    mod = types.ModuleType("antenv.axon_hooks")
    state = {"hook": None}
    mod.set_axon_ntff_profile_hook = lambda h: state.__setitem__("hook", h)
    mod.get_axon_ntff_profile_hook = lambda: state["hook"]
    sys.modules["antenv.axon_hooks"] = mod
    antenv.axon_hooks = mod
    try:
        from trn_agent_boot.trn_boot import _ntff_profile_via_ctypes

        mod.set_axon_ntff_profile_hook(
            _ntff_profile_via_ctypes("/opt/axon/libaxon_pjrt.so")
        )
    except Exception:
        pass


def kernel(x, centers, labels, _profile=False):
    in_maps = shard_inputs(x, centers, labels)
    if _profile:
        _ensure_ntff_hook()
    if "nc" not in _NC_CACHE:
        _NC_CACHE["nc"] = build_nc()
    nc = _NC_CACHE["nc"]
    res = run_bass_kernel_spmd(
        nc, in_maps, list(range(N_CORES)), trace=bool(_profile)
    )
    s = 0.0
    for r in res.results:
        s += float(np.asarray(r["out"], dtype=np.float64).sum())
    val = np.array(np.float32(2.0 - 2.0 * s / PIX_TOTAL))
    if _profile:
        return val, res
    return val


# revision 10
# speedup vs baseline: 1.8135x; 1.2274x over previous
"""CenterLoss kernel for Trainium2 (8 NeuronCores, SPMD data-parallel).

Math: for pixel p with feature x_p (256-ch), label l_p, centers C[19,256]:
    dist_p = ||xn_p||^2 + ||cn_{l_p}||^2 - 2 * xn_p . cn_{l_p}
with xn = x/||x||, cn = C/||C|| (row-wise).  ||xn||^2 == ||cn||^2 == 1 up to
f32 rounding, so   mean(dist) = 2 - (2/B) * S,  S = sum_p (x_p.cn_{l_p})/||x_p||.

Device pipeline (per 2048-px tile, 4 col-tiled 512-px groups g in parallel
strips of the PE array -- every output here is <=32 partitions wide, so
4 matmuls with different moving operands run concurrently):
  - dots4[32g+k, p] = ct.T @ x_g            (PE, strip g, bf16)
  - prodsel4       = onehot4 * dots4        (DVE, one op per tile; onehot4
                     is staged host-side in the same 32g+k partition layout)
  - sel[32g+u, p] += rc-col.T @ prodsel4_g  (PE, strip (g,g); u = tile parity;
                     stationary [19,2] col u = rc = 1/||C_k||)
  - ss[32g+u, p]  += ones-col.T @ xsq_g     (PE, strip g; xsq from ACT Square
                     on chunk 0 / DVE mul on chunk 1)
  - finish per bank (tiles 01 -> bank A, 23 -> B, A overlapped with compute):
    out = sum_p sel * rsqrt(ss)   (ACT Abs_reciprocal_sqrt)

PSUM has_written discipline: start=True clears the WHOLE bank's bits, so
each bank gets exactly one start=True on its first matmul; all later
matmuls use flags=0 (overwrite-if-clear / accumulate-if-set per element).

Sharding: 65536 pixels -> 8 cores x 8192 (core c: image c//2, half c%2).
x channel-major bf16 [2,128,8192]; onehot4 fp8 [4,128,512]; centersT bf16.
Host sums the live partitions (32g+u) of the [2,128,1] output.
"""

import sys

import numpy as np

if "/opt/trn_rl_repo" not in sys.path:
    sys.path.insert(0, "/opt/trn_rl_repo")

import concourse.bacc as bacc
import concourse.tile as tile
from concourse import mybir
from concourse.bass_utils import run_bass_kernel_spmd

N_CORES = 8
C = 256
NCLS = 19
N_IMG, H, W = 4, 128, 128
PIX_TOTAL = N_IMG * H * W          # 65536
PIX_PER_CORE = PIX_TOTAL // N_CORES  # 8192
TILE_F = 2048                      # pixels per DMA tile
N_TILES = PIX_PER_CORE // TILE_F   # 4
GRP = 512                          # pixels per col-strip group
F32 = mybir.dt.float32
BF16 = mybir.dt.bfloat16
FP8 = mybir.dt.float8e4


def build_nc():
    """Build the per-core Bass program (same program on all 8 cores)."""
    AF = mybir.ActivationFunctionType

    nc = bacc.Bacc(None, target_bir_lowering=False, debug=False)
    x_d = nc.dram_tensor("x", [2, 128, PIX_PER_CORE], BF16, kind="ExternalInput")
    oh_d = nc.dram_tensor("onehot4", [N_TILES, 128, GRP], FP8, kind="ExternalInput")
    ct_d = nc.dram_tensor("centersT", [2, 128, NCLS], BF16, kind="ExternalInput")
    out_d = nc.dram_tensor("out", [2, 128, 1], F32, kind="ExternalOutput")

    with tile.TileContext(nc) as tc:
        with (
            tc.tile_pool(name="consts", bufs=1) as consts,
            tc.tile_pool(name="xin", bufs=4) as xin,
            tc.tile_pool(name="xsq", bufs=2) as xsqp,
            tc.tile_pool(name="small", bufs=2) as small,
            tc.tile_pool(name="accum", bufs=1) as accp,
            tc.tile_pool(name="dots", bufs=3, space="PSUM") as dotsp,
            tc.tile_pool(name="acc_ps", bufs=1, space="PSUM") as accps,
        ):
            # ---- DMAs first: x tiles 0,1 then centers then x tiles 2,3 on
            # the sync HWDGE ring; onehot on the gpsimd ring in parallel ----
            xts = []
            for t in range(N_TILES):
                xts.append(
                    xin.tile([128, 2, TILE_F], BF16, tag="xt", name=f"xt{t}")
                )
            for t in (0, 1):
                fsl = slice(t * TILE_F, (t + 1) * TILE_F)
                nc.sync.dma_start(
                    out=xts[t][:], in_=x_d[:, :, fsl].rearrange("a p f -> p a f")
                )
            ct_in = consts.tile([128, 2, NCLS], BF16, tag="ct_in")
            nc.sync.dma_start(out=ct_in[:], in_=ct_d[:].rearrange("a p k -> p a k"))
            for t in (2, 3):
                fsl = slice(t * TILE_F, (t + 1) * TILE_F)
                nc.sync.dma_start(
                    out=xts[t][:], in_=x_d[:, :, fsl].rearrange("a p f -> p a f")
                )
            oh4 = consts.tile([128, N_TILES, GRP], FP8, tag="oh4")
            nc.gpsimd.dma_start(
                out=oh4[:], in_=oh_d[:].rearrange("t p f -> p t f")
            )

            # ---- constants ----
            # wmov first: the PE warmup matmuls depend only on it.
            wmov = consts.tile([128, GRP], BF16, tag="wmov")
            nc.vector.memset(wmov[:], 0.5)
            ctb = consts.tile([128, 2, NCLS], BF16, tag="ctb")
            nc.vector.tensor_copy(ctb[:], ct_in[:])
            ones_b = consts.tile([128, 1], BF16, tag="ones_b")
            nc.vector.memset(ones_b[:], 1.0)
            # ss stationary: sstat[:, u, r] = 1 iff r == u (tile parity u)
            sstat = consts.tile([128, 2, 2], BF16, tag="sstat")
            nc.vector.memset(sstat[:], 0.0)
            for u in range(2):
                nc.vector.memset(sstat[:, u, u : u + 1], 1.0)
            # sel stationary: rcsel[32g+k, u, r] = rc_k iff r == u
            rcsel = consts.tile([128, 2, 2], BF16, tag="rcsel")
            nc.vector.memset(rcsel[:], 0.0)

            # ---- center norms: rc[k] = 1/||C_k|| ----
            csq = consts.tile([128, 2, NCLS], BF16, tag="csq")
            nc.vector.tensor_mul(out=csq[:], in0=ctb[:], in1=ctb[:])
            ssc = accps.tile([NCLS, 1], F32, tag="ssc")
            nc.tensor.matmul(ssc[:], csq[:, 0, :], ones_b[:], start=True, stop=False)
            nc.tensor.matmul(ssc[:], csq[:, 1, :], ones_b[:], start=False, stop=True)
            rc = consts.tile([NCLS, 1], F32, tag="rc")
            nc.scalar.activation(out=rc[:], in_=ssc[:], func=AF.Abs_reciprocal_sqrt)
            for g in range(4):
                for u in range(2):
                    nc.vector.tensor_copy(
                        rcsel[32 * g : 32 * g + NCLS, u, u : u + 1], rc[:]
                    )

            # ---- PE warm-up (HAM unthrottle): independent of any DMA ----
            warm = dotsp.tile([128, GRP], F32, tag="dots")
            for _ in range(12):
                nc.tensor.matmul(
                    warm[0:NCLS, :], wmov[:, 0:NCLS], wmov[:], start=True, stop=True
                )

            # ---- accumulators: bank 0 <- tiles 0,1; bank 1 <- tiles 2,3 ----
            ss_b = [accps.tile([128, GRP], F32, tag=f"ss{b}", name=f"ss{b}") for b in range(2)]
            sel_b = [accps.tile([128, GRP], F32, tag=f"sel{b}", name=f"sel{b}") for b in range(2)]

            pending_sel = []

            def emit_sel(t, ps4):
                b, u = t >> 1, t & 1
                for g in range(4):
                    nc.tensor.matmul(
                        sel_b[b][32 * g : 32 * g + 2, :],
                        rcsel[32 * g : 32 * g + NCLS, u, :],
                        ps4[32 * g : 32 * g + NCLS, :],
                        start=(u == 0 and g == 0),
                        stop=(u == 1 and g == 3),
                        tile_position=(32 * g, 32 * g),
                    )

            def emit_finish(b):
                rsq = accp.tile([128, GRP], F32, tag=f"rsq{b}")
                nc.scalar.activation(
                    out=rsq[:], in_=ss_b[b][:], func=AF.Abs_reciprocal_sqrt
                )
                acc = accp.tile([128, GRP], F32, tag=f"acc{b}")
                nc.vector.tensor_mul(out=acc[:], in0=rsq[:], in1=sel_b[b][:])
                partial = accp.tile([128, 1], F32, tag=f"partial{b}")
                nc.vector.tensor_reduce(
                    out=partial[:],
                    in_=acc[:],
                    axis=mybir.AxisListType.X,
                    op=mybir.AluOpType.add,
                )
                nc.sync.dma_start(out=out_d[b], in_=partial[:])

            # ---- main loop ----
            for t in range(N_TILES):
                b, u = t >> 1, t & 1
                xt = xts[t]
                xsqt = xsqp.tile([128, 2, TILE_F], BF16, tag="xsqt")
                nc.scalar.activation(
                    out=xsqt[:, 0, :], in_=xt[:, 0, :], func=AF.Square
                )
                nc.vector.tensor_mul(
                    out=xsqt[:, 1, :], in0=xt[:, 1, :], in1=xt[:, 1, :]
                )

                dots4 = dotsp.tile([128, GRP], F32, tag="dots")
                for g in range(4):
                    gsl = slice(g * GRP, (g + 1) * GRP)
                    for a in range(2):
                        nc.tensor.matmul(
                            dots4[32 * g : 32 * g + NCLS, :],
                            ctb[:, a, :],
                            xt[:, a, gsl],
                            start=(g == 0 and a == 0),
                            stop=(g == 3 and a == 1),
                            tile_position=(0, 32 * g),
                        )
                ps4 = small.tile([128, GRP], BF16, tag="ps4")
                nc.vector.tensor_mul(out=ps4[:], in0=oh4[:, t, :], in1=dots4[:])
                pending_sel.append((t, ps4))

                for g in range(4):
                    gsl = slice(g * GRP, (g + 1) * GRP)
                    for a in range(2):
                        nc.tensor.matmul(
                            ss_b[b][32 * g : 32 * g + 2, :],
                            sstat[:, u, :],
                            xsqt[:, a, gsl],
                            start=(u == 0 and g == 0 and a == 0),
                            stop=(u == 1 and g == 3 and a == 1),
                            tile_position=(0, 32 * g),
                        )

                # sel for the previous tile (lag 1 so PE never waits on DVE)
                if len(pending_sel) > 1:
                    emit_sel(*pending_sel.pop(0))
                if t == 2:
                    emit_finish(0)  # bank A finish overlaps tiles 2-3
            emit_sel(*pending_sel.pop(0))
            emit_finish(1)

    nc.compile()
    return nc


def shard_inputs(x, centers, labels):
    """Full inputs -> list of 8 per-core input maps."""
    import ml_dtypes

    fp8_np = mybir.dt.np(FP8)
    x = np.asarray(x, dtype=np.float32)
    centers = np.asarray(centers, dtype=np.float32)
    labels = np.asarray(labels)

    xr = x.reshape(N_IMG, C, 2, PIX_PER_CORE)
    labr = labels.reshape(N_IMG, 2, PIX_PER_CORE)
    ctr = np.ascontiguousarray(centers.T.astype(ml_dtypes.bfloat16)).reshape(
        2, 128, NCLS
    )
    kvals = np.arange(NCLS, dtype=np.int64)

    in_maps = []
    for core in range(N_CORES):
        n, j = core // 2, core % 2
        xs = np.ascontiguousarray(
            xr[n, :, j, :].astype(ml_dtypes.bfloat16)
        ).reshape(2, 128, PIX_PER_CORE)
        # onehot4[t, 32g+k, p] = (label[t*2048 + g*512 + p] == k)
        lab4 = labr[n, j].reshape(N_TILES, 4, GRP)
        oh4 = np.zeros((N_TILES, 128, GRP), dtype=fp8_np)
        for g in range(4):
            oh4[:, 32 * g : 32 * g + NCLS, :] = (
                lab4[:, g, None, :] == kvals[None, :, None]
            ).astype(fp8_np)
        in_maps.append(
            {"x": xs, "onehot4": np.ascontiguousarray(oh4), "centersT": ctr}
        )
    return in_maps


_NC_CACHE = {}


def _ensure_ntff_hook():
    """Register the axon NTFF profile hook if the optional antenv.axon_hooks
    module is absent from this image (bass_utils hard-imports it when
    trace=True)."""
    try:
        from antenv.axon_hooks import get_axon_ntff_profile_hook  # noqa: F401

        return
    except ImportError:
        pass
    import types

    import antenv

    mod = types.ModuleType("antenv.axon_hooks")
    state = {"hook": None}
    mod.set_axon_ntff_profile_hook = lambda h: state.__setitem__("hook", h)
    mod.get_axon_ntff_profile_hook = lambda: state["hook"]
    sys.modules["antenv.axon_hooks"] = mod
    antenv.axon_hooks = mod
    try:
        from trn_agent_boot.trn_boot import _ntff_profile_via_ctypes

        mod.set_axon_ntff_profile_hook(
            _ntff_profile_via_ctypes("/opt/axon/libaxon_pjrt.so")
        )
    except Exception:
        pass


# live output partitions: ss/sel rows sit at partition 32g+u
_LIVE = [32 * g + u for g in range(4) for u in range(2)]


def kernel(x, centers, labels, _profile=False):
    in_maps = shard_inputs(x, centers, labels)
    if _profile:
        _ensure_ntff_hook()
    if "nc" not in _NC_CACHE:
        _NC_CACHE["nc"] = build_nc()
    nc = _NC_CACHE["nc"]
    res = run_bass_kernel_spmd(
        nc, in_maps, list(range(N_CORES)), trace=bool(_profile)
    )
    s = 0.0
    for r in res.results:
        o = np.asarray(r["out"], dtype=np.float64)  # [2, 128, 1]
        s += o[:, _LIVE, 0].sum()
    val = np.array(np.float32(2.0 - 2.0 * s / PIX_TOTAL))
    if _profile:
        return val, res
    return val
